# revision 55
# baseline (speedup 1.0000x reference)
"""Causal multi-head attention block (B=2, S=2048, D=1024, H=16) on 8 TRN2 cores.

Sharding: core i handles batch b = i//4 and head group hg = i%4 (4 heads =
256 model dims). Each core computes its heads' attention and a partial
output projection; the host sums the 4 partials per batch and adds b_out.

Per-core device pipeline (bf16 matmuls, fp32 PSUM accumulation):
  1. QKV. Q^T,K^T land as [head_cols, tokens] (lhsT = W, rhs = x^T);
     V lands as [tokens, head_cols] (lhsT = x^T tiles, rhs = W_v) and is
     stored augmented with a ones column so the attention z-matmul also
     produces softmax row sums.
  2. Attention per head, flash-style in the S^T = K.Q^T orientation over
     the causal lower triangle only: S^T[k_tile, q_span] -> exp on ScalarE
     (scale=1/8; no max subtraction, logits are ~N(0,1)) -> P^T bf16 ->
     multiplicative 0/1 mask on the diagonal block -> z^T[d+1, q] +=
     V_aug^T @ P^T accumulated over k tiles in PSUM. Consecutive k tiles
     share one S region so each exp call covers up to 1024 columns.
  3. Normalize as soon as a q-quarter's last k tile lands: recip(rowsum),
     GPSIMD partition-broadcast, z * recip on VectorE -> bf16 zT. The V
     bias is folded into the output bias on the host (b_v @ w_out).
  4. Out-proj: y_partial[t, n] accumulated over the 256 local dims.

Program order is a 4-stage pipeline over 512-token quarters --
QKV(tg0), att(qg0), QKV(tg1), att(qg1), ... out-proj last -- so ScalarE
exp work overlaps PE QKV work and out-proj fills late PE gaps. Host
pre-packs all inputs into SBUF layouts (bf16) for contiguous DMA.
"""

import numpy as np
import ml_dtypes

import concourse.mybir as mybir
import concourse.tile as tile
from concourse import bacc
from concourse.bass_utils import run_bass_kernel_spmd

B = 2
S = 2048
DM = 1024
HD = 64
HLOC = 4                 # heads per core
CLOC = HLOC * HD         # local model dims (256)
MO = DM // 128           # 8 k-subtiles of the model dim
NKT = S // 128           # 16 key tiles


f32 = mybir.dt.float32
bf16 = mybir.dt.bfloat16
EXP = mybir.ActivationFunctionType.Exp

_CACHE = {}


def build(ps_z_bufs=2, pt_bufs=5, op_engines=("dve", "dve", "dve", "mix"),
          interleave_heads=False, dma_splits=(1, 1, 1, 4), sp_bufs=4,
          yp_bufs=4):
    nc = bacc.Bacc("TRN2", target_bir_lowering=False, debug=False)

    xT_d = nc.dram_tensor("xT", [128, MO, S], bf16, kind="ExternalInput")
    wqk_d = nc.dram_tensor("wqk", [128, MO, 2 * CLOC], bf16, kind="ExternalInput")
    wv_d = nc.dram_tensor("wv", [128, MO, CLOC], bf16, kind="ExternalInput")
    wo_d = nc.dram_tensor("wo", [128, 2, DM], bf16, kind="ExternalInput")
    # consts packed as raw bf16 columns: tri[0:128], bq[128:132],
    # bk[132:136], bv[136:144] (f32 values bit-split across bf16 pairs)
    cst_d = nc.dram_tensor("cst", [128, 144], bf16, kind="ExternalInput")
    y_d = nc.dram_tensor("y", [S, DM], f32, kind="ExternalOutput")

    with tile.TileContext(nc) as tc:
        with (
            tc.tile_pool(name="consts", bufs=1) as consts,
            tc.tile_pool(name="acts", bufs=1) as apool,
            tc.tile_pool(name="pt", bufs=pt_bufs) as ppool,
            tc.tile_pool(name="norm", bufs=sp_bufs) as spool,
            tc.tile_pool(name="ycopy", bufs=yp_bufs) as ypool,
            # 8 PSUM banks: ps_s 2x[128,1024]=4 (QKV Q/K + attention S),
            # ps_z [128,512] z accumulators, ps_b rest (V / out-proj)
            tc.tile_pool(name="ps_s", bufs=2, space="PSUM") as ps_s,
            tc.tile_pool(name="ps_z", bufs=ps_z_bufs, space="PSUM") as ps_z,
            tc.tile_pool(name="ps_b", bufs=8 - 4 - ps_z_bufs, space="PSUM") as ps_b,
        ):
            csb = consts.tile([128, 144], bf16)
            wqk = consts.tile([128, MO, 2 * CLOC], bf16)
            wv = consts.tile([128, MO, CLOC], bf16)
            wo = consts.tile([128, 2, DM], bf16)
            xT = apool.tile([128, MO, S], bf16)

            # DMA order = consumption order. First chunks are fine-grained
            # so the first QKV matmul starts ~2.5us in; the rest are big
            # transfers to minimize per-DMA descriptor overhead.
            nc.sync.dma_start(wqk[:, 0:3, 0:256], wqk_d[:, 0:3, 0:256])
            nc.scalar.dma_start(xT[:, 0:3, 0:512], xT_d[:, 0:3, 0:512])
            nc.sync.dma_start(csb[:], cst_d[:])
            nc.sync.dma_start(wqk[:, 3:MO, 0:256], wqk_d[:, 3:MO, 0:256])
            nc.scalar.dma_start(xT[:, 3:MO, 0:512], xT_d[:, 3:MO, 0:512])
            nc.sync.dma_start(wqk[:, :, 256:512], wqk_d[:, :, 256:512])
            nc.scalar.dma_start(wv[:], wv_d[:])
            nc.sync.dma_start(xT[:, :, 512:1024], xT_d[:, :, 512:1024])
            nc.scalar.dma_start(xT[:, :, 1024:1536], xT_d[:, :, 1024:1536])
            nc.sync.dma_start(xT[:, :, 1536:2048], xT_d[:, :, 1536:2048])
            nc.scalar.dma_start(wo[:], wo_d[:])

            tri = csb[:, 0:128]
            bq_sb = csb[:, 128:132].bitcast(f32)
            bk_sb = csb[:, 132:136].bitcast(f32)

            QT = apool.tile([128, 2, S], bf16)
            KT = apool.tile([128, 2, S], bf16)
            # V augmented: [t-part, kt, h, 0:64] = v dims, col 64 = ones
            VA = apool.tile([128, NKT, HLOC, 72], bf16)
            nc.vector.memset(VA[:, :, :, 64:65], 1.0)
            zT = apool.tile([128, 2, S], bf16)

            def emit_qkv_qk(tg, pool=None, ptag="s"):
                pool = pool or ps_s
                tsl = slice(tg * 512, (tg + 1) * 512)
                for ct in range(2):
                    for j, (dst, b_sb) in enumerate(
                        ((QT, bq_sb), (KT, bk_sb))
                    ):
                        csl = slice(ct * 256 + j * 128, ct * 256 + (j + 1) * 128)
                        ps = pool.tile([128, 512 if ptag == "b" else 1024],
                                       f32, tag=ptag,
                                       name=f"qk_{tg}_{ct}_{j}")
                        for mo in range(MO):
                            nc.tensor.matmul(
                                ps[:, 0:512],
                                wqk[:, mo, csl],
                                xT[:, mo, tsl],
                                start=(mo == 0),
                                stop=(mo == MO - 1),
                            )
                        nc.vector.tensor_scalar_add(
                            dst[:, ct, tsl], ps[:, 0:512], b_sb[:, ct : ct + 1]
                        )
            def emit_qkv_v(tg):
                for ti in range(4):
                    tt = tg * 4 + ti
                    ps = ps_b.tile([128, 512], f32, tag="b")
                    for mo in range(MO):
                        nc.tensor.matmul(
                            ps[:, 0:CLOC],
                            xT[:, mo, tt * 128 : (tt + 1) * 128],
                            wv[:, mo, :],
                            start=(mo == 0),
                            stop=(mo == MO - 1),
                        )
                    if tg == 0:
                        nc.scalar.copy(
                            VA[:, tt, :, 0:64],
                            ps[:, 0:CLOC].rearrange("p (h d) -> p h d", d=64),
                        )
                    else:
                        nc.vector.tensor_copy(
                            VA[:, tt, :, 0:64],
                            ps[:, 0:CLOC].rearrange("p (h d) -> p h d", d=64),
                        )

            def emit_attention(qg):
                g0 = qg * 512
                last_kt = 4 * qg + 3

                # pack consecutive k tiles into shared S regions so one
                # exp call covers up to 1024 columns
                groups, cur, cum = [], [], 0
                for kt in range(last_kt + 1):
                    w = g0 + 512 - max(kt * 128, g0)
                    if cum + w > 1024:
                        groups.append(cur)
                        cur, cum = [], 0
                    cur.append((kt, cum, w))
                    cum += w
                groups.append(cur)

                head_lists = ([0, 1, 3, 2] if not interleave_heads
                              else [[0, 1], [2, 3]])
                for hl in head_lists:
                    hs = [hl] if isinstance(hl, int) else hl
                    zp_map = {}
                    for h in hs:
                        zp_map[h] = ps_z.tile([128, 512], f32, tag="z",
                                              name=f"zps_{h}_{qg}")
                    for grp in groups:
                        for h in hs:
                            emit_head_grp(h, grp, zp_map[h], qg, g0, last_kt)
                    for h in hs:
                        emit_norm(h, zp_map[h], qg, g0)

            def emit_head_grp(h, grp, zp, qg, g0, last_kt):
                    hp = (h % 2) * 64
                    ct = h // 2
                    if True:
                        sreg = ps_s.tile([128, 1024], f32, tag="s",
                                         name=f"s_{h}_{qg}_{grp[0][0]}")
                        cum = grp[-1][1] + grp[-1][2]
                        for kt, off, w in grp:
                            q0 = g0 + 512 - w
                            c0 = off
                            while c0 < off + w:
                                cw = min(off + w - c0, 512 - c0 % 512)
                                nc.tensor.matmul(
                                    sreg[:, c0 : c0 + cw],
                                    KT[hp : hp + 64, ct,
                                       kt * 128 : (kt + 1) * 128],
                                    QT[hp : hp + 64, ct,
                                       q0 + c0 - off : q0 + c0 - off + cw],
                                )
                                c0 += cw
                        pT = ppool.tile([128, 1024], bf16, tag="pT")
                        nc.scalar.activation(
                            pT[:, :cum], sreg[:, :cum], EXP, scale=0.125
                        )
                        for kt, off, w in grp:
                            if kt * 128 >= g0:  # diagonal block leads span
                                nc.vector.tensor_mul(
                                    pT[:, off : off + 128],
                                    pT[:, off : off + 128],
                                    tri[:],
                                )
                            q0 = g0 + 512 - w
                            nc.tensor.matmul(
                                zp[0:65, q0 - g0 : 512],
                                VA[:, kt, h, 0:65],
                                pT[:, off : off + w],
                                start=(kt == 0),
                                stop=(kt == last_kt),
                            )

            def emit_norm(h, zp, qg, g0):
                    hp = (h % 2) * 64
                    ct = h // 2
                    rec32 = spool.tile([1, 512], f32, tag="rec32",
                                       name=f"rec_{h}_{qg}")
                    nc.vector.reciprocal(rec32[:], zp[64:65, 0:512])
                    bcast = spool.tile([64, 512], f32, tag="bcast",
                                       name=f"bc_{h}_{qg}")
                    nc.gpsimd.partition_broadcast(bcast[:], rec32[:])
                    # b_v is folded into b_out on the host:
                    # y += (1 (x) b_v) @ w_out is a constant row vector
                    with nc.allow_low_precision(reason="attn out to bf16"):
                        if hp == 0:
                            nc.vector.tensor_mul(
                                zT[0:64, ct, g0 : g0 + 512],
                                zp[0:64, 0:512], bcast[:],
                            )
                        else:
                            zbf = spool.tile([64, 512], bf16, tag="zbf",
                                             name=f"zb_{h}_{qg}")
                            nc.vector.tensor_mul(
                                zbf[:], zp[0:64, 0:512], bcast[:]
                            )
                            nc.sync.dma_start(
                                zT[hp : hp + 64, ct, g0 : g0 + 512], zbf[:]
                            )

            def emit_outproj(qg, copy_eng, dma_split=1):
                for nh in range(2):
                    ysb = ypool.tile([128, 4, 512], f32, tag="y",
                                     name=f"ysb_{qg}_{nh}")
                    nper = 4 // dma_split
                    for ti in range(4):
                        tt = qg * 4 + ti
                        ps = ps_b.tile([128, 512], f32, tag="b")
                        for co in range(2):
                            nc.tensor.matmul(
                                ps[:],
                                zT[:, co, tt * 128 : (tt + 1) * 128],
                                wo[:, co, nh * 512 : (nh + 1) * 512],
                                start=(co == 0),
                                stop=(co == 1),
                            )
                        eng = copy_eng if copy_eng != "mix" else (
                            "act" if (tt + nh) % 2 == 0 else "dve"
                        )
                        if eng == "act":
                            nc.scalar.copy(ysb[:, ti, :], ps[:])
                        else:
                            nc.vector.tensor_copy(ysb[:, ti, :], ps[:])
                        if ti % nper == nper - 1:
                            t0 = tt - nper + 1
                            deng = nc.sync if (ti // nper + nh) % 2 == 0 else nc.scalar
                            deng.dma_start(
                                y_d[t0 * 128 : (tt + 1) * 128,
                                    nh * 512 : (nh + 1) * 512].rearrange(
                                    "(ti p) n -> p ti n", p=128
                                ),
                                ysb[:, ti - nper + 1 : ti + 1, :],
                            )

            # 4-stage software pipeline: attention on quarter qg overlaps
            # the QKV projection of quarter qg+1 on PE
            emit_qkv_qk(0)
            emit_qkv_v(0)
            emit_attention(0)
            emit_qkv_qk(1)
            emit_qkv_v(1)
            emit_attention(1)
            emit_qkv_qk(2)
            emit_qkv_v(2)
            emit_attention(2)
            emit_qkv_qk(3)
            emit_qkv_v(3)
            emit_attention(3)
            for qg in range(4):
                emit_outproj(qg, op_engines[qg], dma_split=dma_splits[qg])

    nc.compile()
    return nc


def _pack_w(w):
    # [DM, C] -> [128, MO, C]: partition p holds rows {mo*128 + p}
    return np.ascontiguousarray(
        w.reshape(MO, 128, w.shape[1]).transpose(1, 0, 2)
    ).astype(ml_dtypes.bfloat16)


def make_in_maps(x, w_qkv, b_qkv, w_out):
    # multiplicative post-exp mask: 1 where k <= q (upper incl diag), else 0
    tri = np.tri(128, 128, 0, dtype=np.float32).T.astype(ml_dtypes.bfloat16)
    in_maps = []
    for core in range(8):
        b = core // 4
        hg = core % 4
        c0 = hg * CLOC
        csl = slice(c0, c0 + CLOC)

        # packed consts: [128, 144] bf16-typed raw columns
        cst = np.zeros((128, 144), np.uint16)
        cst[:, 0:128] = tri.view(np.uint16)
        bq = np.ascontiguousarray(
            b_qkv[csl].astype(np.float32).reshape(2, 128).T
        )
        bk = np.ascontiguousarray(
            b_qkv[DM + c0 : DM + c0 + CLOC].astype(np.float32).reshape(2, 128).T
        )
        bv = np.ascontiguousarray(
            b_qkv[2 * DM + c0 : 2 * DM + c0 + CLOC]
            .astype(np.float32).reshape(HLOC, 64).T
        )
        cst[:, 128:132] = bq.view(np.uint16).reshape(128, 4)
        cst[:, 132:136] = bk.view(np.uint16).reshape(128, 4)
        cst[0:64, 136:144] = bv.view(np.uint16).reshape(64, 8)

        wq_p = _pack_w(w_qkv[:, csl])
        wk_p = _pack_w(w_qkv[:, DM + c0 : DM + c0 + CLOC])
        wqk = np.concatenate(
            [wq_p[:, :, 0:128], wk_p[:, :, 0:128],
             wq_p[:, :, 128:256], wk_p[:, :, 128:256]],
            axis=2,
        )
        in_maps.append(
            {
                "xT": _pack_w(np.ascontiguousarray(x[b].T)),
                "wqk": np.ascontiguousarray(wqk),
                "wv": _pack_w(w_qkv[:, 2 * DM + c0 : 2 * DM + c0 + CLOC]),
                # wo: [CLOC, DM] -> [128, 2, DM]
                "wo": np.ascontiguousarray(
                    w_out[csl, :].reshape(2, 128, DM).transpose(1, 0, 2)
                ).astype(ml_dtypes.bfloat16),
                "cst": cst.view(ml_dtypes.bfloat16),
            }
        )
    return in_maps


def gather(results, b_qkv, w_out, b_out):
    # device skips the V bias; z_norm + b_v projects to a constant row:
    # y += b_v @ w_out, folded into the output bias here
    b_eff = (
        b_out.astype(np.float32)
        + b_qkv[2 * DM :].astype(np.float32) @ w_out.astype(np.float32)
    )
    out = np.empty((B, S, DM), np.float32)
    for b in range(B):
        acc = results[4 * b]["y"].astype(np.float32)
        for j in range(1, 4):
            acc = acc + results[4 * b + j]["y"]
        out[b] = acc + b_eff[None, :]
    return out


def kernel(x, w_qkv, b_qkv, w_out, b_out):
    x = np.asarray(x)
    w_qkv = np.asarray(w_qkv)
    b_qkv = np.asarray(b_qkv)
    w_out = np.asarray(w_out)
    b_out = np.asarray(b_out)

    if "nc" not in _CACHE:
        _CACHE["nc"] = build()
    nc = _CACHE["nc"]

    in_maps = make_in_maps(x, w_qkv, b_qkv, w_out)
    res = run_bass_kernel_spmd(nc, in_maps, core_ids=list(range(8)))
    return gather(res.results, b_qkv, w_out, b_out)
